# revision 31
# baseline (speedup 1.0000x reference)
"""DDiT block kernel for 8 Trainium2 NeuronCores.

Sharding: core c = (b = c//2, half = c%2).  Each core computes the block
output for its (batch, sequence-half): 1024 rows of 2048.  K/V are computed
redundantly for the full sequence on each core (softmax contraction is
permutation invariant, so the host passes rows as [own_half; other_half]).
No collectives.

v2 structure (single fused pipeline, minimal DMA-instruction count):
 - Phase AB: per s-tile: load x -> LN1 (plain) -> one multi-tile DMA
   transpose -> per-dc modulate (scale/shift from AdaLN, bf16 4x) ->
   QKV matmuls -> rope in bf16 (2x DVE) -> one multi-tile transpose to
   qT/kT; V written via ACT copies into an aligned pair-block layout
   carrying the softmax-denominator ones column.
 - Phase C: per (q-chunk, head): scores (64-row matmuls), Exp batched
   over two PSUM banks (N=1024), attn@V with denominator riding at a
   32-aligned partition, normalize via K=1 f32r broadcast matmul.
 - Out-proj (gate folded into W_out columns on host) + residual,
   LN2 + modulate, MLP (gate folded into W2), all emitted so the Tile
   scheduler overlaps them with phase C's ScalarE-bound tail.
Bulk loads/stores ride the idle Pool engine's SWDGE queues.
"""

import os
from contextlib import ExitStack

import numpy as np

B, S, D, H = 4, 2048, 1024, 16
HD = D // H  # 64
J = 4 * D  # 4096
SO = S // 2  # 1024 rows per core
SF = S  # full sequence
P = 128
EPS = 1e-6
N_CORES = 8

ST_F = SF // P  # 16 s-tiles full seq
ST_O = SO // P  # 8 s-tiles own half
DC = D // P  # 8 d-tiles
JT = J // P  # 32 j-tiles
NCH = 512  # matmul free-dim chunk
HP = H // 2  # 8 head pairs
VW = 160  # v_aug pair-block width: [even_v 0:64 | ones 64 | gap | odd_v 96:160]


def _emit(ctx, nc, tc, io):
    import concourse.bass as bass
    import concourse.mybir as mybir

    f32 = mybir.dt.float32
    f32r = mybir.dt.float32r
    bf16 = mybir.dt.bfloat16
    f8 = mybir.dt.float8e4
    DR = mybir.MatmulPerfMode.DoubleRow
    AF = mybir.ActivationFunctionType
    OP = mybir.AluOpType

    def big(shape, dtype, name, side="left"):
        t, free = tc.tile(shape, dtype, name=name, side=side)
        return t, free

    def bcast_row(pool, key, n=D):
        """DMA a [n] dram row into a [P, n] sbuf tile, replicated across partitions."""
        ap_1d = io[key].ap()
        t = pool.tile([P, n], f32, tag=f"row_{key}", name=f"row_{key}", bufs=1)
        src = bass.AP(
            tensor=ap_1d.tensor,
            offset=ap_1d.offset,
            ap=[[0, P], list(ap_1d.ap[0])],
        )
        nc.sync.dma_start(out=t, in_=src)
        return t

    # ---------------- constants / rows ----------------
    pconst = ctx.enter_context(tc.tile_pool(name="const", bufs=1))
    eps_t = pconst.tile([P, 1], f32, name="eps_t")
    nc.vector.memset(eps_t, EPS)
    ones_t = pconst.tile([P, 64], bf16, name="ones_t")
    nc.vector.memset(ones_t, 1.0)

    def col_tile(key, n, tag):
        t = pconst.tile([P, n], f32, tag=tag, name=tag)
        nc.sync.dma_start(t, io[key].ap().rearrange("(a p) -> p a", p=P))
        return t

    a1T = col_tile("a1", DC, "a1T")
    c1T = col_tile("c1", DC, "c1T")
    a2T = col_tile("a2", DC, "a2T")
    c2T = col_tile("c2", DC, "c2T")
    fb1T = col_tile("fb1", JT, "fb1T")

    # ---------------- big persistent tiles ----------------
    wq, free_wq = big([P, DC, 3 * D], f8, "wq")  # 24KB/part
    kT, free_kT = big([P, HP, SF], bf16, "kT", side="right")  # 32KB
    qT, free_qT = big([P, HP, SO], bf16, "qT", side="right")  # 16KB
    v_aug, free_v = big([P, ST_F, H, 65], f8, "v_aug", side="right")  # 16.3KB
    cs_t, free_cs = big([P, 2, ST_F, HD], bf16, "cs_t")  # cos/sin, 4KB
    cos_t = cs_t[:, 0]
    sin_t = cs_t[:, 1]
    nc.sync.dma_start(cos_t, io["cos"].ap().rearrange("(t p) d -> p t d", p=P))
    nc.sync.dma_start(sin_t, io["sin"].ap().rearrange("(t p) d -> p t d", p=P))

    wqkv_r = io["wqkv"].ap().rearrange("(dc p) c -> p dc c", p=P)
    for ch in range(6):
        nc.sync.dma_start(
            wq[:, :, ch * NCH : (ch + 1) * NCH], wqkv_r[:, :, ch * NCH : (ch + 1) * NCH]
        )
    nc.vector.memset(v_aug[:, :, :, 64:65], 1.0)

    def layernorm(pool, x_tile, out_bf):
        """out_bf = (x - mean) * rstd   (plain LN, modulation applied later)."""
        stats = pool.tile([P, 2, 6], f32, tag="bnstats", name="stats")
        mv = pool.tile([P, 2], f32, tag="bnaggr", name="mv")
        xv = x_tile.rearrange("p (g k) -> p g k", g=2)
        for g in range(2):
            nc.vector.bn_stats(out=stats[:, g, :], in_=xv[:, g, :])
        nc.vector.bn_aggr(out=mv, in_=stats)
        rstd = pool.tile([P, 1], f32, tag="rstd", name="rstd")
        nc.scalar.activation(out=rstd, in_=mv[:, 1:2], func=AF.Sqrt, bias=eps_t)
        nc.vector.reciprocal(out=rstd, in_=rstd)
        nc.vector.tensor_scalar(
            out=out_bf, in0=x_tile, scalar1=mv[:, 0:1], scalar2=rstd,
            op0=OP.subtract, op1=OP.mult,
        )

    # ================ phase AB: LN1 + QKV + rope ================
    with tc.tile_pool(name="pab", bufs=3) as pab, \
         tc.tile_pool(name="pabs", bufs=4) as pabs, \
         tc.tile_pool(name="psab", bufs=4, space="PSUM") as psab:
        for st in range(ST_F):
            src = io["x_own"] if st < ST_O else io["x_oth"]
            row0 = (st % ST_O) * P
            x_t = pab.tile([P, D], f32, tag="xin", name="x_t")
            nc.gpsimd.dma_start(x_t, src.ap()[row0 : row0 + P, :])
            y_bf = pab.tile([P, D], bf16, tag="ybf", name="y_bf")
            layernorm(pabs, x_t, y_bf)
            yTt = pab.tile([P, DC, P], bf16, tag="yTt", name="yTt")
            nc.sync.dma_start(out=yTt, in_=y_bf, transpose=True)
            yTm = pab.tile([P, DC, P], f8, tag="yTm", name="yTm")
            for dc in range(DC):
                nc.vector.tensor_scalar(
                    out=yTm[:, dc, :], in0=yTt[:, dc, :],
                    scalar1=a1T[:, dc : dc + 1], scalar2=c1T[:, dc : dc + 1],
                    op0=OP.mult, op1=OP.add,
                )
            chs = range(6) if st < ST_O else range(2, 6)
            for ch in chs:
                ps = psab.tile([P, NCH], f32, tag="qkv", name="qkv_ps")
                for t in range(DC // 2):
                    nc.tensor.matmul(
                        ps,
                        yTm[:, 2 * t : 2 * t + 2, :],
                        wq[:, 2 * t : 2 * t + 2, ch * NCH : (ch + 1) * NCH],
                        start=(t == 0), stop=(t == DC // 2 - 1),
                        perf_mode=DR,
                    )
                if ch < 4:  # Q or K: rope then one multi-tile transpose
                    qbf = pab.tile([P, 8, HD], bf16, tag="qbf", name="qbf")
                    nc.scalar.copy(qbf, ps.rearrange("p (h d) -> p h d", h=8))
                    cos_b = cos_t[:, st, None, :].to_broadcast((P, 8, HD))
                    sin_b = sin_t[:, st, None, :].to_broadcast((P, 8, HD))
                    t1 = pab.tile([P, 8, HD], bf16, tag="ropet1", name="t1")
                    nc.vector.tensor_mul(t1, qbf, cos_b)
                    qshuf = qbf.rearrange("p h (two j) -> p h two j", two=2)[:, :, ::-1, :]
                    t2 = pab.tile([P, 8, HD], bf16, tag="ropet2", name="t2")
                    nc.vector.tensor_mul(
                        t2.rearrange("p h (two j) -> p h two j", two=2),
                        qshuf,
                        sin_b.rearrange("p h (two j) -> p h two j", two=2),
                    )
                    rot = pab.tile([P, NCH], bf16, tag="rot", name="rot")
                    nc.vector.tensor_add(rot.rearrange("p (h d) -> p h d", h=8), t1, t2)
                    dst = qT if ch < 2 else kT
                    pair0 = (ch % 2) * 4
                    nc.sync.dma_start(
                        out=dst[:, pair0 : pair0 + 4, st * P : (st + 1) * P],
                        in_=rot,
                        transpose=True,
                    )
                else:  # V: one ACT copy into the [v|1] per-head layout
                    h0 = (ch - 4) * 8
                    nc.scalar.copy(
                        v_aug[:, st, h0 : h0 + 8, 0:64],
                        ps.rearrange("p (f d) -> p f d", d=HD),
                    )
    free_cs()
    free_wq()
    x_mid, free_xmid = big([P, ST_O, D], bf16, "x_mid")  # 16KB
    y2Tm, free_y2Tm = big([P, DC, SO], f8, "y2Tm")  # 8KB
    attnT, free_attnT = big([P, HP, SO], bf16, "attnT")  # 16KB
    wout_t, free_wout = big([P, DC, D], bf16, "wout_t")  # 16KB

    # ================ phase C: attention (+ out-proj, LN2 interleaved) ======
    with tc.tile_pool(name="pc", bufs=4) as pc, \
         tc.tile_pool(name="pcs", bufs=2) as pcs, \
         tc.tile_pool(name="pd", bufs=2) as pd, \
         tc.tile_pool(name="pds", bufs=4) as pds, \
         tc.tile_pool(name="ps_sc", bufs=2, space="PSUM") as ps_sc, \
         tc.tile_pool(name="ps_av", bufs=2, space="PSUM") as ps_av, \
         tc.tile_pool(name="ps_bc", bufs=1, space="PSUM") as ps_bc, \
         tc.tile_pool(name="ps_op", bufs=1, space="PSUM") as ps_op:
        nc.sync.dma_start(wout_t, io["wout"].ap().rearrange("(dc p) c -> p dc c", p=P))
        for qc in range(2):
            qs = slice(qc * NCH, (qc + 1) * NCH)
            for h in range(H):
                hp, odd = h // 2, h % 2
                rows = slice(64, 128) if odd else slice(0, 64)
                psU = ps_av.tile([P, NCH], f32, tag="attnv", name="psU")
                for ktp in range(ST_F // 2):
                    ps2 = ps_sc.tile([P, 2, NCH], f32, tag="scores", name="ps2")
                    for i in range(2):
                        kt = 2 * ktp + i
                        nc.tensor.matmul(
                            ps2[:, i, :],
                            kT[rows, hp, kt * P : (kt + 1) * P],
                            qT[rows, hp, qs],
                            start=True, stop=True,
                        )
                    prb = pc.tile([P, 2, NCH], f8, tag="probs", name="prb")
                    nc.scalar.activation(
                        out=prb, in_=ps2, func=AF.Exp, scale=0.125 / 4096.0
                    )
                    nc.tensor.matmul(
                        psU[0:65],
                        v_aug[:, 2 * ktp : 2 * ktp + 2, h, :],
                        prb,
                        start=(ktp == 0), stop=(ktp == ST_F // 2 - 1),
                        perf_mode=DR,
                    )
                rec = pcs.tile([P, NCH], bf16, tag="rec", name="rec")
                with nc.allow_low_precision(reason="softmax denom broadcast in bf16"):
                    nc.vector.reciprocal(rec[64:65, :], psU[64:65, :])
                bc = ps_bc.tile([P, NCH], f32, tag="bc", name="bc")
                nc.tensor.matmul(
                    bc[0:64], ones_t[64:65, 0:64], rec[64:65, :],
                    start=True, stop=True,
                )
                bcs = pcs.tile([P, NCH], f32, tag="bcs", name="bcs")
                nc.vector.tensor_copy(bcs[0:64], bc[0:64])
                if not odd:
                    nc.vector.tensor_mul(
                        attnT[0:64, hp, qs], psU[0:64], bcs[0:64]
                    )
                else:
                    tmpB = pcs.tile([P, NCH], bf16, tag="tmpB", name="tmpB")
                    nc.vector.tensor_mul(tmpB[0:64, :], psU[0:64], bcs[0:64])
                    nc.sync.dma_start(attnT[64:128, hp, qs], tmpB[0:64, :])
            # out-proj + residual + LN2 for this q-chunk's four s-tiles
            for st in range(qc * 4, qc * 4 + 4):
                x_t = pd.tile([P, D], f32, tag="x4", name="x_t4")
                nc.gpsimd.dma_start(x_t, io["x_own"].ap()[st * P : (st + 1) * P, :])
                for c in range(D // NCH):
                    cs = slice(c * NCH, (c + 1) * NCH)
                    ps = ps_op.tile([P, NCH], f32, tag="opj", name="op_ps")
                    for dc in range(DC):
                        nc.tensor.matmul(
                            ps,
                            attnT[:, dc, st * P : (st + 1) * P],
                            wout_t[:, dc, cs],
                            start=(dc == 0), stop=(dc == DC - 1),
                        )
                    nc.vector.tensor_add(x_mid[:, st, cs], ps, x_t[:, cs])
                y2bf = pd.tile([P, D], bf16, tag="y2bf", name="y2bf")
                layernorm(pds, x_mid[:, st, :], y2bf)
                y2Tt = pd.tile([P, DC, P], bf16, tag="y2Tt", name="y2Tt")
                nc.sync.dma_start(out=y2Tt, in_=y2bf, transpose=True)
                for dc in range(DC):
                    nc.vector.tensor_scalar(
                        out=y2Tm[:, dc, st * P : (st + 1) * P], in0=y2Tt[:, dc, :],
                        scalar1=a2T[:, dc : dc + 1], scalar2=c2T[:, dc : dc + 1],
                        op0=OP.mult, op1=OP.add,
                    )
    free_wout()
    free_attnT()
    free_v()
    free_qT()
    free_kT()

    # ================ MLP ================
    hT, free_hT = big([P, JT, SO], f8, "hT")  # 32KB
    w2sb, free_w2 = big([P, JT, D], f8, "w2sb", side="right")  # 32KB
    with tc.tile_pool(name="pf", bufs=4) as pf, \
         tc.tile_pool(name="pg", bufs=3) as pg, \
         tc.tile_pool(name="ps_f1", bufs=3, space="PSUM") as ps_f1, \
         tc.tile_pool(name="ps_f2", bufs=2, space="PSUM") as ps_f2:
        gb2r = bcast_row(pg, "gb2")
        nc.sync.dma_start(w2sb, io["w2"].ap().rearrange("(o p) c -> p o c", p=P))
        w1_r = io["w1"].ap().rearrange("(dc p) c -> p dc c", p=P)
        for sc in range(SO // NCH):
            ss = slice(sc * NCH, (sc + 1) * NCH)
            for jt in range(JT):
                w1_t = pf.tile([P, DC, P], f8, tag="w1", name="w1_t")
                nc.sync.dma_start(w1_t, w1_r[:, :, jt * P : (jt + 1) * P])
                ps = ps_f1.tile([P, NCH], f32, tag="fc1", name="fc1_ps")
                for t in range(DC // 2):
                    nc.tensor.matmul(
                        ps,
                        w1_t[:, 2 * t : 2 * t + 2, :],
                        y2Tm[:, 2 * t : 2 * t + 2, ss],
                        start=(t == 0), stop=(t == DC // 2 - 1),
                        perf_mode=DR,
                    )
                nc.scalar.activation(
                    out=hT[:, jt, ss], in_=ps,
                    func=AF.Gelu_apprx_tanh, bias=fb1T[:, jt : jt + 1],
                    scale=1.0 / 64.0,
                )
            # fc2 + final residual for this half's four s-tiles
            for st in range(sc * 4, sc * 4 + 4):
                o_t = pg.tile([P, D], f32, tag="out", name="o_t")
                for c in range(D // NCH):
                    cs = slice(c * NCH, (c + 1) * NCH)
                    ps = ps_f2.tile([P, NCH], f32, tag="fc2", name="fc2_ps")
                    for t in range(JT // 2):
                        nc.tensor.matmul(
                            ps,
                            hT[:, 2 * t : 2 * t + 2, st * P : (st + 1) * P],
                            w2sb[:, 2 * t : 2 * t + 2, cs],
                            start=(t == 0), stop=(t == JT // 2 - 1),
                            perf_mode=DR,
                        )
                    ft = pg.tile([P, NCH], f32, tag="fin_t", name="fin_t")
                    nc.vector.scalar_tensor_tensor(
                        out=ft, in0=ps, scalar=1.0 / 64.0, in1=gb2r[:, cs],
                        op0=OP.mult, op1=OP.add,
                    )
                    nc.vector.tensor_add(o_t[:, cs], ft, x_mid[:, st, cs])
                nc.gpsimd.dma_start(io["out"].ap()[st * P : (st + 1) * P, :], o_t)
    free_hT()
    free_w2()
    free_y2Tm()
    free_xmid()


def build_nc():
    import concourse.tile as tile
    import concourse.mybir as mybir
    from concourse import bacc

    f32 = mybir.dt.float32
    bf16 = mybir.dt.bfloat16
    f8 = mybir.dt.float8e4

    nc = bacc.Bacc("TRN2", target_bir_lowering=False, debug=False)
    io = {}
    io["x_own"] = nc.dram_tensor("x_own", [SO, D], f32, kind="ExternalInput")
    io["x_oth"] = nc.dram_tensor("x_oth", [SO, D], f32, kind="ExternalInput")
    io["cos"] = nc.dram_tensor("cos", [SF, HD], bf16, kind="ExternalInput")
    io["sin"] = nc.dram_tensor("sin", [SF, HD], bf16, kind="ExternalInput")
    io["wqkv"] = nc.dram_tensor("wqkv", [D, 3 * D], f8, kind="ExternalInput")
    io["wout"] = nc.dram_tensor("wout", [D, D], bf16, kind="ExternalInput")
    io["w1"] = nc.dram_tensor("w1", [D, J], f8, kind="ExternalInput")
    io["w2"] = nc.dram_tensor("w2", [J, D], f8, kind="ExternalInput")
    for name in ["a1", "c1", "a2", "c2", "gb2"]:
        io[name] = nc.dram_tensor(name, [D], f32, kind="ExternalInput")
    io["fb1"] = nc.dram_tensor("fb1", [J], f32, kind="ExternalInput")
    io["out"] = nc.dram_tensor("out", [SO, D], f32, kind="ExternalOutput")

    with tile.TileContext(nc) as tc:
        with ExitStack() as ctx:
            _emit(ctx, nc, tc, io)
    nc.finalize()
    return nc


def host_prep(inputs):
    """Build the 8 per-core input maps from the full problem inputs."""
    import ml_dtypes

    bf = ml_dtypes.bfloat16
    x = np.asarray(inputs["x"], np.float32)
    sigma_emb = np.asarray(inputs["sigma_emb"], np.float32)
    ada = sigma_emb @ np.asarray(inputs["ada_W"], np.float32) + np.asarray(
        inputs["ada_b"], np.float32
    )
    ada = ada.reshape(B, 6, D)
    shift_msa, scale_msa, gate_msa, shift_mlp, scale_mlp, gate_mlp = (
        ada[:, i] for i in range(6)
    )
    ln1_s = np.asarray(inputs["ln1_scale"], np.float32)
    ln1_b = np.asarray(inputs["ln1_bias"], np.float32)
    ln2_s = np.asarray(inputs["ln2_scale"], np.float32)
    ln2_b = np.asarray(inputs["ln2_bias"], np.float32)

    a1 = ln1_s[None] * (1.0 + scale_msa)  # [B, D]
    c1 = ln1_b[None] * (1.0 + scale_msa) + shift_msa
    a2 = ln2_s[None] * (1.0 + scale_mlp)
    c2 = ln2_b[None] * (1.0 + scale_mlp) + shift_mlp
    gb2 = gate_mlp * np.asarray(inputs["fc2_b"], np.float32)[None]

    # rope tables (match reference)
    inv_freq = 1.0 / (10000.0 ** (np.arange(0, HD, 2, dtype=np.float32) / HD))
    t = np.arange(S, dtype=np.float32)
    freqs = np.einsum("n,d->nd", t, inv_freq)
    emb = np.concatenate([freqs, freqs], axis=-1)  # [S, HD]
    cos = np.cos(emb).astype(np.float32)
    sin = np.sin(emb).astype(np.float32)
    sin_signed = sin.copy()
    sin_signed[:, : HD // 2] *= -1.0  # fold rotate_half sign

    f8 = ml_dtypes.float8_e4m3
    # fp8 weights scaled by 64 to clear the e4m3 denormal range; the
    # kernel folds 1/64 back in (gelu scale, fc2 epilogue, wout columns).
    wqkv = (np.asarray(inputs["W_qkv"], np.float32) * 64.0).astype(f8)
    wout_f = np.asarray(inputs["W_out"], np.float32) / 64.0
    w1 = (np.asarray(inputs["fc1_W"], np.float32) * 64.0).astype(f8)
    w2_f = np.asarray(inputs["fc2_W"], np.float32) * 64.0
    fb1 = np.asarray(inputs["fc1_b"], np.float32)

    in_maps = []
    for c in range(N_CORES):
        b, h = c // 2, c % 2
        own = slice(h * SO, (h + 1) * SO)
        oth = slice((1 - h) * SO, (2 - h) * SO)
        in_maps.append(
            {
                "x_own": np.ascontiguousarray(x[b, own]),
                "x_oth": np.ascontiguousarray(x[b, oth]),
                "cos": np.concatenate([cos[own], cos[oth]], 0).astype(bf),
                "sin": np.concatenate(
                    [sin_signed[own], sin_signed[oth]], 0
                ).astype(bf),
                "wqkv": wqkv,
                # attention gate folded into out-proj columns
                "wout": (wout_f * gate_msa[b][None, :]).astype(bf),
                "w1": w1,
                # mlp gate folded into fc2 columns
                "w2": (w2_f * gate_mlp[b][None, :]).astype(f8),
                "a1": np.ascontiguousarray(a1[b]),
                "c1": np.ascontiguousarray(c1[b]),
                "a2": np.ascontiguousarray(a2[b]),
                "c2": np.ascontiguousarray(c2[b]),
                "gb2": np.ascontiguousarray(gb2[b]),
                "fb1": fb1,
            }
        )
    return in_maps


_NC_CACHE = {}


def kernel(**inputs) -> np.ndarray:
    import sys

    if "/opt/trn_rl_repo" not in sys.path:
        sys.path.insert(0, "/opt/trn_rl_repo")
    from concourse.bass_utils import run_bass_kernel_spmd

    in_maps = host_prep(inputs)
    if "nc" not in _NC_CACHE:
        _NC_CACHE["nc"] = build_nc()
    nc = _NC_CACHE["nc"]
    res = run_bass_kernel_spmd(
        nc,
        in_maps,
        core_ids=list(range(N_CORES)),
        trace=bool(int(os.environ.get("KERNEL_TRACE", "0"))),
    )
    out = np.empty((B, S, D), np.float32)
    for c in range(N_CORES):
        b, h = c // 2, c % 2
        out[b, h * SO : (h + 1) * SO] = res.results[c]["out"]
    _NC_CACHE["last_result"] = res
    return out
